# revision 34
# baseline (speedup 1.0000x reference)
"""LocallyConnected2D Trainium2 kernel.

Problem: out[b,o,h,w] = sum_{c,kh,kw} xpad[b,c,h+kh,w+kw] * W[(c,kh,kw), (h,w), o] + bias[o,h,w]
  B=16, C_IN=32, H=W=64, C_OUT=64, KH=KW=3, pad=1  ->  DEPTH=288, S=4096.

Sharding: S split into 8 contiguous blocks of 512 (8 output rows each), one per
core; full batch per core; no cross-core reduction.  The bias add (pure
elementwise on the output) runs on the host during unshard.

Per-core algorithm (HBM/weight-stream bound; weights read exactly once):
  - contraction d=(c,kh,kw) regrouped into 3 chunks by kh, each K=96 rows
    ordered (kw, c).  The stationary matmul operand for chunk kh at output
    location s=(h,w) is xs3[0:96, b] = x[c, b, h+kh, w+kw]: a single strided
    AP into SBUF tensor xs3 holding 3 kw-shifted replicas of the transposed
    input.  Only replica 0 is DMA'd from HBM; replicas 1,2 are built by
    shifted SBUF->SBUF DMAs on the ACT ring (so the SP ring FIFO never
    stalls and block 0's weights land immediately).
  - the PE runs in 128x32 column-tiled mode: 4 consecutive locations are
    assigned to the 4 column tiles (tile_position (0,32j)), so their
    3-matmul accumulation chains (K=96, N=64 each) execute CONCURRENTLY,
    writing PSUM partitions 32j..32j+15.  A PSUM bank holds 8 slots x 4
    tiles = 32 locations.
  - weights stream in s-blocks of 64 over three descriptor queues (SP-HWDGE
    64-row pieces, SWDGE 32-row pieces); HWDGE deals strided transfers one
    pow2-rows chunk (<=32KB) per SDMA engine and dumps remainders on the
    first engine, so every transfer is 16*2^k rows (xs dram rows are
    stride-padded to keep the source strided).
  - PSUM banks are evacuated with full-width [128,512] DVE copies to a bf16
    stage tile and DMA'd out as out[j*16+b, blk*1024 + q*64 + o] where the
    location within the shard is s_local = blk*64 + q*4 + j; the host adds
    bias and casts/transposes to (B, C_OUT, H, W) fp32.
"""

import numpy as np
import ml_dtypes

BF16 = ml_dtypes.bfloat16

# ---------------- problem constants (hardcoded; kernel.py must be self-contained) ---
B = 16
C_IN = 32
H = W = 64
C_OUT = 64
KH = KW = 3
S = H * W                     # 4096
N_CORES = 8
S_SH = S // N_CORES           # 512 output locations per core
ROWS_SH = S_SH // W           # 8 output rows per core
IN_ROWS = ROWS_SH + 2         # 10 padded input rows per core
WPAD = W + 2                  # 66
XS_F = B * IN_ROWS * WPAD     # 10560 free elements of xs
K1 = KW * C_IN                # 96  contraction rows per kh chunk
SBW = 64                      # weight-stream block size (locations per block)
NBLK = S_SH // SBW            # 8 blocks
NT = 4                        # column tiles (locations running concurrently)
QB = SBW // NT                # 16 location-groups per block
PSUM_Q = 8                    # groups per PSUM bank (8*64 = 512 fp32)

TRACE = False                 # test.py sets True to get an NTFF profile
LAST_RESULTS = None           # BassKernelResults of the last run (for test.py)

_CACHE = {}


def _build_nc():
    import concourse.mybir as mybir
    from concourse import bacc
    from concourse.tile import TileContext

    fp32 = mybir.dt.float32
    bf16 = mybir.dt.bfloat16
    nc = bacc.Bacc(None)

    # free dim padded by 64 so the DMA source rows are strided (HWDGE does
    # not split contiguous sources across SDMA engines).
    xs_d = nc.dram_tensor("xs", [C_IN, XS_F + 64], bf16, kind="ExternalInput")
    # weights are stored BLOCK-MAJOR (row blk*96+d) with a 32-element row pad:
    # each SDMA engine's 4-row chunk is then ~contiguous in DRAM (sequential
    # HBM reads), while the pad keeps the source strided so HWDGE still
    # splits it across engines.
    wk_d = [
        nc.dram_tensor(f"wk{kh}", [NBLK * K1, SBW * C_OUT + 32], bf16,
                       kind="ExternalInput")
        for kh in range(KH)
    ]
    out_d = nc.dram_tensor(
        "out", [NT * B, NBLK * QB * C_OUT], bf16, kind="ExternalOutput"
    )

    with TileContext(nc) as tc:
        with (
            tc.tile_pool(name="xs3", bufs=1) as xs3_pool,
            tc.tile_pool(name="wk", bufs=7) as wk_pool,
            tc.tile_pool(name="stage", bufs=4) as stage_pool,
            tc.tile_pool(name="psum", bufs=8, space="PSUM") as psum_pool,
        ):
            # xs3: rows 32*kw+c = input channel c shifted kw elements left.
            xs3 = xs3_pool.tile([K1, XS_F], bf16)
            # HBM -> SBUF: replica 0 split across both HWDGE rings.
            nc.sync.dma_start(out=xs3[0:16, :], in_=xs_d[0:16, 0:XS_F])
            nc.scalar.dma_start(out=xs3[16:32, :], in_=xs_d[16:32, 0:XS_F])
            # replicas kw=1,2: shifted SBUF->SBUF copies (no HBM traffic) in
            # 16-row pieces (no remainder -> no first-engine dump), one
            # replica per ring so they build in parallel.
            nc.scalar.dma_start(out=xs3[32:48, 0:XS_F - 1], in_=xs3[0:16, 1:XS_F])
            nc.scalar.dma_start(out=xs3[48:64, 0:XS_F - 1], in_=xs3[16:32, 1:XS_F])
            nc.sync.dma_start(out=xs3[64:80, 0:XS_F - 2], in_=xs3[0:16, 2:XS_F])
            nc.sync.dma_start(out=xs3[80:96, 0:XS_F - 2], in_=xs3[16:32, 2:XS_F])

            # view of xs3 as [p, b, f] where f = h*66 + w
            xs3r = xs3[:].rearrange("p (b f) -> p b f", b=B)

            for blk in range(NBLK):
                s0 = blk * SBW
                wkt = [
                    wk_pool.tile([K1, SBW * C_OUT], bf16, tag=f"wk{kh}",
                                 name=f"wk{kh}t_{blk}")
                    for kh in range(KH)
                ]
                r0 = blk * K1
                for kh in range(KH):
                    nc.sync.dma_start(
                        out=wkt[kh][0:64, :],
                        in_=wk_d[kh][r0:r0 + 64, 0:SBW * C_OUT],
                    )
                    nc.gpsimd.dma_start(
                        out=wkt[kh][64:96, :],
                        in_=wk_d[kh][r0 + 64:r0 + 96, 0:SBW * C_OUT],
                    )

                stage = stage_pool.tile([128, 2 * PSUM_Q * C_OUT], bf16)
                for bi in range(QB // PSUM_Q):          # 2 banks per block
                    ps = psum_pool.tile([128, PSUM_Q * C_OUT], fp32)
                    for q8 in range(PSUM_Q):
                        q = bi * PSUM_Q + q8            # group within block
                        for kh in range(KH):
                            for j in range(NT):
                                sl = q * NT + j         # location within block
                                s = s0 + sl             # location within shard
                                h, w = divmod(s, W)
                                lhsT = xs3r[0:K1, :, (h + kh) * WPAD + w]
                                rhs = wkt[kh][0:K1, sl * C_OUT:(sl + 1) * C_OUT]
                                nc.tensor.matmul(
                                    ps[32 * j:32 * j + B,
                                       q8 * C_OUT:(q8 + 1) * C_OUT],
                                    lhsT,
                                    rhs,
                                    start=(kh == 0),
                                    stop=(kh == 2),
                                    tile_position=(0, 32 * j),
                                )
                    nc.vector.tensor_copy(
                        stage[:, bi * PSUM_Q * C_OUT:(bi + 1) * PSUM_Q * C_OUT],
                        ps[:, :],
                    )
                for j in range(NT):
                    eng = nc.scalar if j % 2 == 0 else nc.sync
                    eng.dma_start(
                        out=out_d[j * B:(j + 1) * B,
                                  blk * QB * C_OUT:(blk + 1) * QB * C_OUT],
                        in_=stage[32 * j:32 * j + B, :],
                    )
    return nc


def _prep_inputs(x, weights):
    """Host-side shard + regather.  Returns list of 8 in_maps."""
    x = np.ascontiguousarray(x, dtype=np.float32)
    w = np.ascontiguousarray(weights, dtype=np.float32).reshape(
        C_IN, KH, KW, S, C_OUT
    )

    xp = np.zeros((B, C_IN, H + 2, WPAD), dtype=np.float32)
    xp[:, :, 1:H + 1, 1:W + 1] = x
    xs_all = xp.transpose(1, 0, 2, 3)  # (c, b, h, w)

    in_maps = []
    for i in range(N_CORES):
        r0 = i * ROWS_SH
        xs_c = np.ascontiguousarray(xs_all[:, :, r0:r0 + IN_ROWS, :]).reshape(C_IN, XS_F)
        # rows 0-31: channels, kw=0 replica (kw=1,2 built on-chip)
        xs1 = np.zeros((C_IN, XS_F + 64), dtype=np.float32)
        xs1[:, 0:XS_F] = xs_c
        s0 = i * S_SH
        m = {"xs": xs1.astype(BF16)}
        for kh in range(KH):
            wk = w[:, kh, :, s0:s0 + S_SH, :].transpose(1, 0, 2, 3)  # (kw, c, 512, 64)
            wk = np.ascontiguousarray(wk).reshape(K1, S_SH * C_OUT)
            # block-major + 32-element row pad (see _build_nc)
            wkb = np.zeros((NBLK * K1, SBW * C_OUT + 32), dtype=np.float32)
            for blk in range(NBLK):
                wkb[blk * K1:(blk + 1) * K1, 0:SBW * C_OUT] = (
                    wk[:, blk * SBW * C_OUT:(blk + 1) * SBW * C_OUT]
                )
            m[f"wk{kh}"] = wkb.astype(BF16)
        in_maps.append(m)
    return in_maps


def kernel(x, weights, bias):
    global LAST_RESULTS
    from concourse.bass_utils import run_bass_kernel_spmd

    if "nc" not in _CACHE:
        nc = _build_nc()
        if not nc.is_finalized():
            nc.finalize()
        _CACHE["nc"] = nc
    nc = _CACHE["nc"]

    in_maps = _prep_inputs(x, weights)
    res = run_bass_kernel_spmd(
        nc, in_maps, core_ids=list(range(N_CORES)), trace=TRACE
    )
    LAST_RESULTS = res

    out = np.empty((B, C_OUT, H, W), dtype=np.float32)
    for i in range(N_CORES):
        # out_d[j*16+b, blk*1024 + q*64 + o], s_local = blk*64 + q*4 + j
        oc = res.results[i]["out"].astype(np.float32)
        oc = oc.reshape(NT, B, NBLK, QB, C_OUT)         # [j, b, blk, q, o]
        oc = oc.transpose(1, 4, 2, 3, 0).reshape(B, C_OUT, ROWS_SH, W)
        out[:, :, i * ROWS_SH:(i + 1) * ROWS_SH, :] = oc
    out += np.asarray(bias, dtype=np.float32)  # bias add on host
    return out
